# revision 33
# baseline (speedup 1.0000x reference)
"""Segment-reduce contrastive loss kernel for Trainium2 (8 NeuronCores).

Strategy (data-parallel over batch, per the sharding hint):
  - Each of the 8 cores gets one batch element. Per-class partial sums
    are computed on device; the host sums the 8 cores' partials (the
    "all-reduce"), normalizes, and does the tiny 19x19 contrastive
    logsumexp in numpy.
  - Features are staged as ONE packed fp8 DRAM image in pixel-major
    layout: per 128-pixel group g, cols [g*1024, g*1024+1024) hold
    [s 512 | t 512] with element [p, ...] belonging to pixel g*128+p.
    Pixels sit on the partition dim, so each per-class segment sum is a
    single one-hot matmul. fp8_e4m3 staging quarters HBM traffic vs
    fp32 (~16.8 MB/core); PSUM accumulation stays fp32 and one-hots are
    exact in fp8, so the only precision loss is input rounding (loss
    rel-err 1.66e-3 vs the fp32 reference; the gate is 2e-2).
  - The DRAM row stride is padded to 133680 (= 16*8355): a 2^17 stride
    put all 128 rows of a chunk on the same HBM bank pattern and cut
    the per-engine DMA rate from ~26.4 to ~21.5 GB/s.
  - One-hots are NOT streamed: labels ride in a tiny f32 side tensor
    (iota row 0..18 | 128 label columns) and all 128 one-hot [128,19]
    fp8 tiles are generated by ONE DVE is_equal with broadcast access
    patterns (step-0 dims). DVE is otherwise idle; this saves ~1.9% of
    HBM traffic, which is the binding roofline (16 SDMA engines x
    ~26.4 GB/s/engine ~= 42 GB/ms, 16.8 MB => ~40us stream floor).
  - Chunks of 16 groups => 16 KB descriptors (the fastest measured
    size; 8 KB ~26.2, 32 KB slower and high-variance). SBUF holds the
    WHOLE stream (131 KB/partition of 224), so every chunk gets a
    dedicated tile and all chunk DMA triggers are issued up-front with
    no buffer-recycling semaphore round-trips, alternating the two
    HWDGE queues. The tail is two 4-group chunks, one per queue, so
    both queues finish together (a finer taper measured slower: tiny
    tail chunks serialize behind the drain triggers in the queue FIFO).
  - The one-hot matmuls use only 19 PE columns, so four run
    concurrently via col-tiling: (tensor, group-parity) pairs target
    distinct 32-column groups / PSUM banks. PE is far from binding
    (~28us busy over a ~45us stream).
  - Drain: the even pair of accumulators shares one PSUM bank and the
    odd pair another (disjoint partition bands), so each pair drains
    with a SINGLE wide copy (cast to fp16 -- sums are O(100), fp16
    rel-err ~1e-4). Evens stop one group early and drain while the
    last group streams; the odd copy (DVE, ~0.7us) is the only
    serial-tail compute after the final matmul. 2 x [64,512] fp16
    DMAs; small partition-sliced DMAs are NOT sprayed across engines
    (a [19,512] DMA ran serially on one engine), 64 rows spray fine.

Known variance: run-to-run, SDMA engine 15 (sometimes 0) -- the engine
owning the last/first 8-row block of every chunk DMA -- can run ~20%
slow, adding up to ~8us (observed 57.8-67us; median ~60-64). Diagnosed
as HBM contention correlated with the paired NeuronCore, NOT
allocation-edge adjacency: 16 dead rows of padding at both tensor ends
did not remove it (the padding is kept as cheap insurance). Row-sliced
sub-DMAs to spread those rows measured far worse (every engine's
descriptor rate collapsed ~40%). Per-core chunk-order rotation via
register-driven dynamic DRAM offsets (de-lockstepping the cores'
address sequences; labels permuted to match on host) was implemented
and CORRECT but did not suppress the slow mode either and cost ~1us
on fast rolls (bounds-check + reg_loads), so it was reverted. The
contention is apparently tied to engine/port identity rather than
address alignment; a per-core 2-row data stagger (dynamic row-base
offsets breaking cross-core band congruence) was also correct but
measured WORSE (65.7-73.2us), confirming it is not address-congruence
either. Left as-is.
"""

import sys

for _p in ("/opt/trn_rl_repo",):
    if _p not in sys.path:
        sys.path.insert(0, _p)

from contextlib import ExitStack

import ml_dtypes
import numpy as np

import concourse.bass as bass
import concourse.mybir as mybir
from concourse import bacc, tile
from concourse.bass_utils import run_bass_kernel_spmd

NUM_CLASSES = 19
TEMP = 0.1
EPS = 1e-12

B, C, H, W = 8, 512, 128, 128
HW = H * W
N_CORES = 8
P = 128
NG = HW // P  # 128 pixel groups of 128
F32 = mybir.dt.float32
F16 = mybir.dt.float16
LABDT = mybir.dt.float32  # DVE per-partition scalar operands must be f32

QDT = mybir.dt.float8e4
QDT_NP = ml_dtypes.float8_e4m3

# Chunk sizes in 128-pixel groups. One 8-group head chunk (8KB
# descriptors keep the DGE-ring ramp efficient); 16-group chunks (16KB
# descriptors, the fastest measured size) for the body; two 4-group
# tail chunks (one per queue, finishing together). A [4,2,1,1] taper measured ~3us SLOWER:
# the tiny chunks' descriptors land at the very end of each queue's
# FIFO behind the drain triggers and serialize the tail.
SIZES = [8] + [16] * 7 + [4, 4]
assert sum(SIZES) == NG
GOFF = np.concatenate(([0], np.cumsum(SIZES))).tolist()

FEAT_W = NG * 2 * C  # 131072 fp8 bytes per partition
# DRAM row stride must NOT be a power of two: with stride 2^17 all 128
# partition rows of a chunk map to the same HBM bank pattern and the
# per-engine DMA rate drops from ~26.2 to ~21.5 GB/s. Pad to 133680
# (= 16 * 8355, max pow-2 factor 16) — the pad bytes are never read.
FEAT_STRIDE = 133680
# 16 dead rows at each end of the DRAM image: SDMA gives the first/last
# 8-row blocks of every chunk DMA to engines 0/15, and when the
# allocator lands a paired core's buffer adjacently those edge
# addresses run ~20% slow (the +8us straggler). With the real data at
# rows [16:144] of a 160-row tensor, the allocation-edge hot zones
# (~1MB each) contain only pad bytes that are never read.
PAD_ROWS = 16
FEAT_ROWS = PAD_ROWS + P + PAD_ROWS
LABW = NUM_CLASSES + NG  # iota cols 0..18 | labels col per group


def build_nc():
    nc = bacc.Bacc()
    labi = nc.declare_dram_parameter("labi", [P, LABW], LABDT, isOutput=False)
    feat = nc.declare_dram_parameter("feat", [FEAT_ROWS, FEAT_STRIDE], QDT, isOutput=False)
    out = nc.declare_dram_parameter("sums", [P, C], F16, isOutput=True)

    with ExitStack() as ctx:
        tc = ctx.enter_context(tile.TileContext(nc))
        sb = ctx.enter_context(tc.tile_pool(name="sb", bufs=1))
        acc_pool = ctx.enter_context(tc.tile_pool(name="acc", bufs=1, space="PSUM"))

        # Labels+iota first on the scalar queue (tiny; also warms the ring).
        # high_priority so the Tile scheduler doesn't push it behind the
        # feature chunk DMAs (the one-hots gate the whole matmul chain).
        labt = sb.tile([P, LABW], LABDT, tag="labt", name="labt")
        oh = sb.tile([P, NG * NUM_CLASSES], QDT, tag="oh", name="oh")
        with tc.high_priority():
            nc.scalar.dma_start(labt[:], labi[:])
            # One-hot gen on DVE in ONE broadcast-AP op:
            #   oh[p, g*19+k] = (iota[p, k] == lab[p, g])
            iota_b = labt[:, 0:NUM_CLASSES].unsqueeze(1).to_broadcast(
                [P, NG, NUM_CLASSES]
            )
            lab_b = labt[:, NUM_CLASSES:LABW].unsqueeze(2).to_broadcast(
                [P, NG, NUM_CLASSES]
            )
            oh_3d = bass.AP(
                oh.tensor,
                oh.offset,
                [[oh.ap[0][0], P], [NUM_CLASSES, NG], [1, NUM_CLASSES]],
            )
            nc.vector.tensor_tensor(oh_3d, iota_b, lab_b, mybir.AluOpType.is_equal)

        # All 13 chunk DMAs issued up-front, alternating the two HWDGE
        # queues. Dedicated tiles: no recycling, no mid-stream semaphores.
        # One whole-chunk DMA per chunk: SDMA gives engine e rows
        # [8e, 8e+8) (contiguous relative blocks). Splitting chunks into
        # row-sliced sub-DMAs to spread the boundary rows measured FAR
        # slower (every engine's per-descriptor rate collapsed ~40%), so
        # whole-chunk DMAs it is.
        nats = []
        for j, s in enumerate(SIZES):
            nt = sb.tile([P, s * 2 * C], QDT, tag=f"nat{j}", name=f"nat_{j}")
            dmae = nc.scalar if j % 2 == 0 else nc.sync
            dmae.dma_start(
                nt[:],
                feat[
                    PAD_ROWS : PAD_ROWS + P,
                    GOFF[j] * 2 * C : GOFF[j + 1] * 2 * C,
                ],
            )
            nats.append(nt)

        # Col-group cg = 2*(g%2) + (0:s, 1:t) writes PSUM partitions
        # [32*cg, 32*cg+19). The even pair (cg 0,1) shares ONE PSUM bank
        # and the odd pair (cg 2,3) another — disjoint partition ranges,
        # so each pair drains with a single wide copy instead of two.
        acc01 = acc_pool.tile([P, C], F32, tag="acc01", name="acc01")
        acc23 = acc_pool.tile([P, C], F32, tag="acc23", name="acc23")
        acc = [acc01, acc01, acc23, acc23]
        ob = sb.tile([P, C], F16, tag="ob", name="ob")
        sl = [slice(32 * cg, 32 * cg + NUM_CLASSES) for cg in range(4)]

        g = 0
        for j, s in enumerate(SIZES):
            nt = nats[j]
            for gl in range(s):
                ohg = oh[:, g * NUM_CLASSES : (g + 1) * NUM_CLASSES]
                par = g % 2
                for ti in range(2):
                    cg = 2 * par + ti
                    fo = gl * 2 * C + ti * C
                    nc.tensor.matmul(
                        acc[cg][sl[cg], :],
                        ohg,
                        nt[:, fo : fo + C],
                        start=(g == par),
                        stop=(g == NG - 2 + par),
                        tile_position=(0, 32 * cg),
                    )
                g += 1
                if g == NG - 1:
                    # Even-pair accumulators just stopped: drain + store
                    # them now (one wide ACT copy spanning both 19-row
                    # bands), overlapping the final group's DMA/matmuls.
                    # ob is fp16 (copies cast): halves the out-DMA bytes.
                    nc.scalar.copy(ob[0:51, :], acc01[0:51, :])
                    nc.sync.dma_start(out[0:64, :], ob[0:64, :])
        # Odd-pair drain right after the final matmuls is the critical
        # tail: split by column halves across DVE+ACT (parallel ~0.4us
        # copies), then store each half via its own queue in parallel.
        nc.vector.tensor_copy(ob[64:115, 0:256], acc23[64:115, 0:256])
        nc.scalar.copy(ob[64:115, 256:512], acc23[64:115, 256:512])
        nc.sync.dma_start(out[64:128, 0:256], ob[64:128, 0:256])
        nc.scalar.dma_start(out[64:128, 256:512], ob[64:128, 256:512])
    nc.finalize()
    return nc


_NC_CACHE = None


def _get_nc():
    global _NC_CACHE
    if _NC_CACHE is None:
        _NC_CACHE = build_nc()
    return _NC_CACHE


def _pack_core(fs_i, ft_i, lab_i):
    """Pack one batch element: fp8 pixel-major feature image + f32
    iota|labels side tensor (partition = pixel % 128)."""
    sT = fs_i.reshape(C, NG, P).astype(QDT_NP).transpose(2, 1, 0)  # [P, NG, C]
    tT = ft_i.reshape(C, NG, P).astype(QDT_NP).transpose(2, 1, 0)
    img = np.zeros((FEAT_ROWS, FEAT_STRIDE), QDT_NP)
    img[PAD_ROWS : PAD_ROWS + P, :FEAT_W] = np.stack([sT, tT], axis=2).reshape(
        P, FEAT_W
    )

    labi = np.empty((P, LABW), np.float32)
    labi[:, :NUM_CLASSES] = np.arange(NUM_CLASSES, dtype=np.float32)
    labi[:, NUM_CLASSES:] = lab_i.reshape(NG, P).T.astype(np.float32)
    return {"feat": img, "labi": labi}


def _make_in_maps(features_s, features_t, labels):
    return [
        _pack_core(features_s[i], features_t[i], labels[i].reshape(-1))
        for i in range(N_CORES)
    ]


def _finish_on_host(results, labels):
    S_s = np.zeros((NUM_CLASSES, C), np.float64)
    S_t = np.zeros((NUM_CLASSES, C), np.float64)
    for r in results:
        o = r["sums"]
        S_s += o[0:NUM_CLASSES]
        S_s += o[64 : 64 + NUM_CLASSES]
        S_t += o[32 : 32 + NUM_CLASSES]
        S_t += o[96 : 96 + NUM_CLASSES]
    counts = np.bincount(
        labels.reshape(-1), minlength=NUM_CLASSES
    ).astype(np.float64)
    denom = np.maximum(counts, 1.0)[:, None]

    def l2n(x):
        n = np.linalg.norm(x, axis=1, keepdims=True)
        return x / np.maximum(n, EPS)

    logits = (l2n(S_s / denom) @ l2n(S_t / denom).T) / TEMP
    m = logits.max(axis=1, keepdims=True)
    lse = m[:, 0] + np.log(np.exp(logits - m).sum(axis=1))
    per_class = np.diag(logits) - lse
    present = counts > 0
    loss = -np.sum(np.where(present, per_class, 0.0)) / np.sum(present)
    return np.asarray(loss, dtype=np.float32)


def kernel(features_s, features_t, labels, _trace=False):
    features_s = np.asarray(features_s, dtype=np.float32)
    features_t = np.asarray(features_t, dtype=np.float32)
    labels = np.asarray(labels)
    nc = _get_nc()
    in_maps = _make_in_maps(features_s, features_t, labels)
    res = run_bass_kernel_spmd(nc, in_maps, list(range(N_CORES)), trace=_trace)
    loss = _finish_on_host(res.results, labels)
    if _trace:
        return loss, res
    return loss


# revision 34
# speedup vs baseline: 1.0212x; 1.0212x over previous
"""Segment-reduce contrastive loss kernel for Trainium2 (8 NeuronCores).

Strategy (data-parallel over batch, per the sharding hint):
  - Each of the 8 cores gets one batch element. Per-class partial sums
    are computed on device; the host sums the 8 cores' partials (the
    "all-reduce"), normalizes, and does the tiny 19x19 contrastive
    logsumexp in numpy.
  - Features are staged as ONE packed fp8 DRAM image in pixel-major
    layout: per 128-pixel group g, cols [g*1024, g*1024+1024) hold
    [s 512 | t 512] with element [p, ...] belonging to pixel g*128+p.
    Pixels sit on the partition dim, so each per-class segment sum is a
    single one-hot matmul. fp8_e4m3 staging quarters HBM traffic vs
    fp32 (~16.8 MB/core); PSUM accumulation stays fp32 and one-hots are
    exact in fp8, so the only precision loss is input rounding (loss
    rel-err 1.66e-3 vs the fp32 reference; the gate is 2e-2).
  - The DRAM row stride is padded to 133680 (= 16*8355): a 2^17 stride
    put all 128 rows of a chunk on the same HBM bank pattern and cut
    the per-engine DMA rate from ~26.4 to ~21.5 GB/s.
  - One-hots are NOT streamed: labels ride in a tiny f32 side tensor
    (iota row 0..18 | 128 label columns) and all 128 one-hot [128,19]
    fp8 tiles are generated by ONE DVE is_equal with broadcast access
    patterns (step-0 dims). DVE is otherwise idle; this saves ~1.9% of
    HBM traffic, which is the binding roofline (16 SDMA engines x
    ~26.4 GB/s/engine ~= 42 GB/ms, 16.8 MB => ~40us stream floor).
  - Chunks of 16 groups => 16 KB descriptors (the fastest measured
    size; 8 KB ~26.2, 32 KB slower and high-variance). SBUF holds the
    WHOLE stream (131 KB/partition of 224), so every chunk gets a
    dedicated tile and all chunk DMA triggers are issued up-front with
    no buffer-recycling semaphore round-trips, alternating the two
    HWDGE queues. The tail is two 4-group chunks, one per queue, so
    both queues finish together (a finer taper measured slower: tiny
    tail chunks serialize behind the drain triggers in the queue FIFO).
  - The one-hot matmuls use only 19 PE columns, so four run
    concurrently via col-tiling: (tensor, group-parity) pairs target
    distinct 32-column groups / PSUM banks. PE is far from binding
    (~28us busy over a ~45us stream).
  - Drain: the even pair of accumulators shares one PSUM bank and the
    odd pair another (disjoint partition bands), so each pair drains
    with a SINGLE wide copy (cast to fp16 -- sums are O(100), fp16
    rel-err ~1e-4). Evens stop one group early and drain while the
    last group streams; the odd copy (DVE, ~0.7us) is the only
    serial-tail compute after the final matmul. 2 x [64,512] fp16
    DMAs; small partition-sliced DMAs are NOT sprayed across engines
    (a [19,512] DMA ran serially on one engine), 64 rows spray fine.

Known variance: run-to-run, SDMA engine 15 (sometimes 0) -- the engine
owning the last/first 8-row block of every chunk DMA -- can run ~20%
slow, adding up to ~8us (observed 57.8-67us; median ~60-64). Diagnosed
as HBM contention correlated with the paired NeuronCore, NOT
allocation-edge adjacency: 16 dead rows of padding at both tensor ends
did not remove it (the padding is kept as cheap insurance). Row-sliced
sub-DMAs to spread those rows measured far worse (every engine's
descriptor rate collapsed ~40%). Per-core chunk-order rotation via
register-driven dynamic DRAM offsets (de-lockstepping the cores'
address sequences; labels permuted to match on host) was implemented
and CORRECT but did not suppress the slow mode either and cost ~1us
on fast rolls (bounds-check + reg_loads), so it was reverted. The
contention is apparently tied to engine/port identity rather than
address alignment; a per-core 2-row data stagger (dynamic row-base
offsets breaking cross-core band congruence) was also correct but
measured WORSE (65.7-73.2us), confirming it is not address-congruence
either. Left as-is.
"""

import sys

for _p in ("/opt/trn_rl_repo",):
    if _p not in sys.path:
        sys.path.insert(0, _p)

from contextlib import ExitStack

import ml_dtypes
import numpy as np

import concourse.bass as bass
import concourse.mybir as mybir
from concourse import bacc, tile
from concourse.bass_utils import run_bass_kernel_spmd

NUM_CLASSES = 19
TEMP = 0.1
EPS = 1e-12

B, C, H, W = 8, 512, 128, 128
HW = H * W
N_CORES = 8
P = 128
NG = HW // P  # 128 pixel groups of 128
F32 = mybir.dt.float32
F16 = mybir.dt.float16
LABDT = mybir.dt.float32  # DVE per-partition scalar operands must be f32

QDT = mybir.dt.float8e4
QDT_NP = ml_dtypes.float8_e4m3

# Chunk sizes in 128-pixel groups. Small head chunks so the stream ramps
# while the DGE rings spin up; 16-group chunks (16KB descriptors, the
# fastest measured size) for the body; two 4-group tail chunks (one per
# queue, finishing together). A [4,2,1,1] taper measured ~3us SLOWER:
# the tiny chunks' descriptors land at the very end of each queue's
# FIFO behind the drain triggers and serialize the tail.
SIZES = [2, 6] + [16] * 7 + [4, 4]
assert sum(SIZES) == NG
GOFF = np.concatenate(([0], np.cumsum(SIZES))).tolist()

FEAT_W = NG * 2 * C  # 131072 fp8 bytes per partition
# DRAM row stride must NOT be a power of two: with stride 2^17 all 128
# partition rows of a chunk map to the same HBM bank pattern and the
# per-engine DMA rate drops from ~26.2 to ~21.5 GB/s. Pad to 133680
# (= 16 * 8355, max pow-2 factor 16) — the pad bytes are never read.
FEAT_STRIDE = 133680
# 16 dead rows at each end of the DRAM image: SDMA gives the first/last
# 8-row blocks of every chunk DMA to engines 0/15, and when the
# allocator lands a paired core's buffer adjacently those edge
# addresses run ~20% slow (the +8us straggler). With the real data at
# rows [16:144] of a 160-row tensor, the allocation-edge hot zones
# (~1MB each) contain only pad bytes that are never read.
PAD_ROWS = 16
FEAT_ROWS = PAD_ROWS + P + PAD_ROWS
LABW = NUM_CLASSES + NG  # iota cols 0..18 | labels col per group


def build_nc():
    nc = bacc.Bacc()
    labi = nc.declare_dram_parameter("labi", [P, LABW], LABDT, isOutput=False)
    feat = nc.declare_dram_parameter("feat", [FEAT_ROWS, FEAT_STRIDE], QDT, isOutput=False)
    out = nc.declare_dram_parameter("sums", [P, C], F16, isOutput=True)

    with ExitStack() as ctx:
        tc = ctx.enter_context(tile.TileContext(nc))
        sb = ctx.enter_context(tc.tile_pool(name="sb", bufs=1))
        acc_pool = ctx.enter_context(tc.tile_pool(name="acc", bufs=1, space="PSUM"))

        # Labels+iota first on the scalar queue (tiny; also warms the ring).
        # high_priority so the Tile scheduler doesn't push it behind the
        # feature chunk DMAs (the one-hots gate the whole matmul chain).
        labt = sb.tile([P, LABW], LABDT, tag="labt", name="labt")
        oh = sb.tile([P, NG * NUM_CLASSES], QDT, tag="oh", name="oh")
        with tc.high_priority():
            nc.scalar.dma_start(labt[:], labi[:])
            # One-hot gen on DVE in ONE broadcast-AP op:
            #   oh[p, g*19+k] = (iota[p, k] == lab[p, g])
            iota_b = labt[:, 0:NUM_CLASSES].unsqueeze(1).to_broadcast(
                [P, NG, NUM_CLASSES]
            )
            lab_b = labt[:, NUM_CLASSES:LABW].unsqueeze(2).to_broadcast(
                [P, NG, NUM_CLASSES]
            )
            oh_3d = bass.AP(
                oh.tensor,
                oh.offset,
                [[oh.ap[0][0], P], [NUM_CLASSES, NG], [1, NUM_CLASSES]],
            )
            nc.vector.tensor_tensor(oh_3d, iota_b, lab_b, mybir.AluOpType.is_equal)

        # All 13 chunk DMAs issued up-front, alternating the two HWDGE
        # queues. Dedicated tiles: no recycling, no mid-stream semaphores.
        # One whole-chunk DMA per chunk: SDMA gives engine e rows
        # [8e, 8e+8) (contiguous relative blocks). Splitting chunks into
        # row-sliced sub-DMAs to spread the boundary rows measured FAR
        # slower (every engine's per-descriptor rate collapsed ~40%), so
        # whole-chunk DMAs it is.
        nats = []
        for j, s in enumerate(SIZES):
            nt = sb.tile([P, s * 2 * C], QDT, tag=f"nat{j}", name=f"nat_{j}")
            dmae = nc.scalar if j % 2 == 0 else nc.sync
            dmae.dma_start(
                nt[:],
                feat[
                    PAD_ROWS : PAD_ROWS + P,
                    GOFF[j] * 2 * C : GOFF[j + 1] * 2 * C,
                ],
            )
            nats.append(nt)

        # Col-group cg = 2*(g%2) + (0:s, 1:t) writes PSUM partitions
        # [32*cg, 32*cg+19). The even pair (cg 0,1) shares ONE PSUM bank
        # and the odd pair (cg 2,3) another — disjoint partition ranges,
        # so each pair drains with a single wide copy instead of two.
        acc01 = acc_pool.tile([P, C], F32, tag="acc01", name="acc01")
        acc23 = acc_pool.tile([P, C], F32, tag="acc23", name="acc23")
        acc = [acc01, acc01, acc23, acc23]
        ob = sb.tile([P, C], F16, tag="ob", name="ob")
        sl = [slice(32 * cg, 32 * cg + NUM_CLASSES) for cg in range(4)]

        g = 0
        for j, s in enumerate(SIZES):
            nt = nats[j]
            for gl in range(s):
                ohg = oh[:, g * NUM_CLASSES : (g + 1) * NUM_CLASSES]
                par = g % 2
                for ti in range(2):
                    cg = 2 * par + ti
                    fo = gl * 2 * C + ti * C
                    nc.tensor.matmul(
                        acc[cg][sl[cg], :],
                        ohg,
                        nt[:, fo : fo + C],
                        start=(g == par),
                        stop=(g == NG - 2 + par),
                        tile_position=(0, 32 * cg),
                    )
                g += 1
                if g == NG - 1:
                    # Even-pair accumulators just stopped: drain + store
                    # them now (one wide ACT copy spanning both 19-row
                    # bands), overlapping the final group's DMA/matmuls.
                    # ob is fp16 (copies cast): halves the out-DMA bytes.
                    nc.scalar.copy(ob[0:51, :], acc01[0:51, :])
                    nc.sync.dma_start(out[0:64, :], ob[0:64, :])
        # Odd-pair drain right after the final matmuls: one wide DVE
        # copy (0.7us, the critical tail), then one [64,512] fp16 DMA.
        nc.vector.tensor_copy(ob[64:115, :], acc23[64:115, :])
        nc.scalar.dma_start(out[64:128, :], ob[64:128, :])
    nc.finalize()
    return nc


_NC_CACHE = None


def _get_nc():
    global _NC_CACHE
    if _NC_CACHE is None:
        _NC_CACHE = build_nc()
    return _NC_CACHE


def _pack_core(fs_i, ft_i, lab_i):
    """Pack one batch element: fp8 pixel-major feature image + f32
    iota|labels side tensor (partition = pixel % 128)."""
    sT = fs_i.reshape(C, NG, P).astype(QDT_NP).transpose(2, 1, 0)  # [P, NG, C]
    tT = ft_i.reshape(C, NG, P).astype(QDT_NP).transpose(2, 1, 0)
    img = np.zeros((FEAT_ROWS, FEAT_STRIDE), QDT_NP)
    img[PAD_ROWS : PAD_ROWS + P, :FEAT_W] = np.stack([sT, tT], axis=2).reshape(
        P, FEAT_W
    )

    labi = np.empty((P, LABW), np.float32)
    labi[:, :NUM_CLASSES] = np.arange(NUM_CLASSES, dtype=np.float32)
    labi[:, NUM_CLASSES:] = lab_i.reshape(NG, P).T.astype(np.float32)
    return {"feat": img, "labi": labi}


def _make_in_maps(features_s, features_t, labels):
    return [
        _pack_core(features_s[i], features_t[i], labels[i].reshape(-1))
        for i in range(N_CORES)
    ]


def _finish_on_host(results, labels):
    S_s = np.zeros((NUM_CLASSES, C), np.float64)
    S_t = np.zeros((NUM_CLASSES, C), np.float64)
    for r in results:
        o = r["sums"]
        S_s += o[0:NUM_CLASSES]
        S_s += o[64 : 64 + NUM_CLASSES]
        S_t += o[32 : 32 + NUM_CLASSES]
        S_t += o[96 : 96 + NUM_CLASSES]
    counts = np.bincount(
        labels.reshape(-1), minlength=NUM_CLASSES
    ).astype(np.float64)
    denom = np.maximum(counts, 1.0)[:, None]

    def l2n(x):
        n = np.linalg.norm(x, axis=1, keepdims=True)
        return x / np.maximum(n, EPS)

    logits = (l2n(S_s / denom) @ l2n(S_t / denom).T) / TEMP
    m = logits.max(axis=1, keepdims=True)
    lse = m[:, 0] + np.log(np.exp(logits - m).sum(axis=1))
    per_class = np.diag(logits) - lse
    present = counts > 0
    loss = -np.sum(np.where(present, per_class, 0.0)) / np.sum(present)
    return np.asarray(loss, dtype=np.float32)


def kernel(features_s, features_t, labels, _trace=False):
    features_s = np.asarray(features_s, dtype=np.float32)
    features_t = np.asarray(features_t, dtype=np.float32)
    labels = np.asarray(labels)
    nc = _get_nc()
    in_maps = _make_in_maps(features_s, features_t, labels)
    res = run_bass_kernel_spmd(nc, in_maps, list(range(N_CORES)), trace=_trace)
    loss = _finish_on_host(res.results, labels)
    if _trace:
        return loss, res
    return loss


# revision 36
# speedup vs baseline: 1.1526x; 1.1287x over previous
"""Segment-reduce contrastive loss kernel for Trainium2 (8 NeuronCores).

Strategy (data-parallel over batch, per the sharding hint):
  - Each of the 8 cores gets one batch element. Per-class partial sums
    are computed on device; the host sums the 8 cores' partials (the
    "all-reduce"), normalizes, and does the tiny 19x19 contrastive
    logsumexp in numpy.
  - Features are staged as ONE packed fp8 DRAM image in pixel-major
    layout: per 128-pixel group g, cols [g*1024, g*1024+1024) hold
    [s 512 | t 512] with element [p, ...] belonging to pixel g*128+p.
    Pixels sit on the partition dim, so each per-class segment sum is a
    single one-hot matmul. fp8_e4m3 staging quarters HBM traffic vs
    fp32 (~16.8 MB/core); PSUM accumulation stays fp32 and one-hots are
    exact in fp8, so the only precision loss is input rounding (loss
    rel-err 1.66e-3 vs the fp32 reference; the gate is 2e-2).
  - The DRAM row stride is padded to 133680 (= 16*8355): a 2^17 stride
    put all 128 rows of a chunk on the same HBM bank pattern and cut
    the per-engine DMA rate from ~26.4 to ~21.5 GB/s.
  - One-hots are NOT streamed: labels ride in a tiny f32 side tensor
    (iota row 0..18 | 128 label columns) and all 128 one-hot [128,19]
    fp8 tiles are generated by ONE DVE is_equal with broadcast access
    patterns (step-0 dims). DVE is otherwise idle; this saves ~1.9% of
    HBM traffic, which is the binding roofline (16 SDMA engines x
    ~26.4 GB/s/engine ~= 42 GB/ms, 16.8 MB => ~40us stream floor).
  - Chunks of 16 groups => 16 KB descriptors (the fastest measured
    size; 8 KB ~26.2, 32 KB slower and high-variance). SBUF holds the
    WHOLE stream (131 KB/partition of 224), so every chunk gets a
    dedicated tile and all chunk DMA triggers are issued up-front with
    no buffer-recycling semaphore round-trips, alternating the two
    HWDGE queues. The tail is two 4-group chunks, one per queue, so
    both queues finish together (a finer taper measured slower: tiny
    tail chunks serialize behind the drain triggers in the queue FIFO).
  - The one-hot matmuls use only 19 PE columns, so four run
    concurrently via col-tiling: (tensor, group-parity) pairs target
    distinct 32-column groups / PSUM banks. PE is far from binding
    (~28us busy over a ~45us stream).
  - Drain: the even pair of accumulators shares one PSUM bank and the
    odd pair another (disjoint partition bands), so each pair drains
    with a SINGLE wide copy (cast to fp16 -- sums are O(100), fp16
    rel-err ~1e-4). Evens stop one group early and drain while the
    last group streams; the odd copy (DVE, ~0.7us) is the only
    serial-tail compute after the final matmul. 2 x [64,512] fp16
    DMAs; small partition-sliced DMAs are NOT sprayed across engines
    (a [19,512] DMA ran serially on one engine), 64 rows spray fine.

Known variance: run-to-run, SDMA engine 15 (sometimes 0) -- the engine
owning the last/first 8-row block of every chunk DMA -- can run ~20%
slow, adding up to ~8us (observed 57.8-67us; median ~60-64). Diagnosed
as HBM contention correlated with the paired NeuronCore, NOT
allocation-edge adjacency: 16 dead rows of padding at both tensor ends
did not remove it (the padding is kept as cheap insurance). Row-sliced
sub-DMAs to spread those rows measured far worse (every engine's
descriptor rate collapsed ~40%). Per-core chunk-order rotation via
register-driven dynamic DRAM offsets (de-lockstepping the cores'
address sequences; labels permuted to match on host) was implemented
and CORRECT but did not suppress the slow mode either and cost ~1us
on fast rolls (bounds-check + reg_loads), so it was reverted. The
contention is apparently tied to engine/port identity rather than
address alignment; a per-core 2-row data stagger (dynamic row-base
offsets breaking cross-core band congruence) was also correct but
measured WORSE (65.7-73.2us), confirming it is not address-congruence
either. Left as-is.
"""

import sys

for _p in ("/opt/trn_rl_repo",):
    if _p not in sys.path:
        sys.path.insert(0, _p)

from contextlib import ExitStack

import ml_dtypes
import numpy as np

import concourse.bass as bass
import concourse.mybir as mybir
from concourse import bacc, tile
from concourse.bass_utils import run_bass_kernel_spmd

NUM_CLASSES = 19
TEMP = 0.1
EPS = 1e-12

B, C, H, W = 8, 512, 128, 128
HW = H * W
N_CORES = 8
P = 128
NG = HW // P  # 128 pixel groups of 128
F32 = mybir.dt.float32
F16 = mybir.dt.float16
LABDT = mybir.dt.float32  # DVE per-partition scalar operands must be f32

QDT = mybir.dt.float8e4
QDT_NP = ml_dtypes.float8_e4m3

# Chunk sizes in 128-pixel groups. Small head chunks so the stream ramps
# while the DGE rings spin up; 16-group chunks (16KB descriptors, the
# fastest measured size) for the body; two 4-group tail chunks (one per
# queue, finishing together). A [4,2,1,1] taper measured ~3us SLOWER:
# the tiny chunks' descriptors land at the very end of each queue's
# FIFO behind the drain triggers and serialize the tail.
SIZES = [2, 6] + [16] * 7 + [4, 4]
assert sum(SIZES) == NG
GOFF = np.concatenate(([0], np.cumsum(SIZES))).tolist()

FEAT_W = NG * 2 * C  # 131072 fp8 bytes per partition
# DRAM row stride must NOT be a power of two: with stride 2^17 all 128
# partition rows of a chunk map to the same HBM bank pattern and the
# per-engine DMA rate drops from ~26.2 to ~21.5 GB/s. Pad to 133680
# (= 16 * 8355, max pow-2 factor 16) — the pad bytes are never read.
FEAT_STRIDE = 133680
# 16 dead rows at each end of the DRAM image: SDMA gives the first/last
# 8-row blocks of every chunk DMA to engines 0/15, and when the
# allocator lands a paired core's buffer adjacently those edge
# addresses run ~20% slow (the +8us straggler). With the real data at
# rows [16:144] of a 160-row tensor, the allocation-edge hot zones
# (~1MB each) contain only pad bytes that are never read.
PAD_ROWS = 16
FEAT_ROWS = PAD_ROWS + P + PAD_ROWS
LABW = NUM_CLASSES + NG  # iota cols 0..18 | labels col per group


def build_nc():
    nc = bacc.Bacc()
    labi = nc.declare_dram_parameter("labi", [P, LABW], LABDT, isOutput=False)
    feat = nc.declare_dram_parameter("feat", [FEAT_ROWS, FEAT_STRIDE], QDT, isOutput=False)
    out = nc.declare_dram_parameter("sums", [P, C], F16, isOutput=True)

    with ExitStack() as ctx:
        tc = ctx.enter_context(tile.TileContext(nc))
        sb = ctx.enter_context(tc.tile_pool(name="sb", bufs=1))
        acc_pool = ctx.enter_context(tc.tile_pool(name="acc", bufs=1, space="PSUM"))

        # Labels+iota first on the scalar queue (tiny; also warms the ring).
        # high_priority so the Tile scheduler doesn't push it behind the
        # feature chunk DMAs (the one-hots gate the whole matmul chain).
        labt = sb.tile([P, LABW], LABDT, tag="labt", name="labt")
        oh = sb.tile([P, NG * NUM_CLASSES], QDT, tag="oh", name="oh")
        with tc.high_priority():
            nc.scalar.dma_start(labt[:], labi[:])
            # One-hot gen on DVE in ONE broadcast-AP op:
            #   oh[p, g*19+k] = (iota[p, k] == lab[p, g])
            iota_b = labt[:, 0:NUM_CLASSES].unsqueeze(1).to_broadcast(
                [P, NG, NUM_CLASSES]
            )
            lab_b = labt[:, NUM_CLASSES:LABW].unsqueeze(2).to_broadcast(
                [P, NG, NUM_CLASSES]
            )
            oh_3d = bass.AP(
                oh.tensor,
                oh.offset,
                [[oh.ap[0][0], P], [NUM_CLASSES, NG], [1, NUM_CLASSES]],
            )
            nc.vector.tensor_tensor(oh_3d, iota_b, lab_b, mybir.AluOpType.is_equal)

        # All 13 chunk DMAs issued up-front, alternating the two HWDGE
        # queues. Dedicated tiles: no recycling, no mid-stream semaphores.
        # One whole-chunk DMA per chunk: SDMA gives engine e rows
        # [8e, 8e+8) (contiguous relative blocks). Splitting chunks into
        # row-sliced sub-DMAs to spread the boundary rows measured FAR
        # slower (every engine's per-descriptor rate collapsed ~40%), so
        # whole-chunk DMAs it is.
        nats = []
        for j, s in enumerate(SIZES):
            nt = sb.tile([P, s * 2 * C], QDT, tag=f"nat{j}", name=f"nat_{j}")
            dmae = nc.scalar if j % 2 == 0 else nc.sync
            dmae.dma_start(
                nt[:],
                feat[
                    PAD_ROWS : PAD_ROWS + P,
                    GOFF[j] * 2 * C : GOFF[j + 1] * 2 * C,
                ],
            )
            nats.append(nt)

        # Col-group cg = 2*(g%2) + (0:s, 1:t) writes PSUM partitions
        # [32*cg, 32*cg+19). The even pair (cg 0,1) shares ONE PSUM bank
        # and the odd pair (cg 2,3) another — disjoint partition ranges,
        # so each pair drains with a single wide copy instead of two.
        acc01 = acc_pool.tile([P, C], F32, tag="acc01", name="acc01")
        acc23 = acc_pool.tile([P, C], F32, tag="acc23", name="acc23")
        acc = [acc01, acc01, acc23, acc23]
        ob = sb.tile([P, C], F16, tag="ob", name="ob")
        sl = [slice(32 * cg, 32 * cg + NUM_CLASSES) for cg in range(4)]

        g = 0
        for j, s in enumerate(SIZES):
            nt = nats[j]
            for gl in range(s):
                ohg = oh[:, g * NUM_CLASSES : (g + 1) * NUM_CLASSES]
                par = g % 2
                for ti in range(2):
                    cg = 2 * par + ti
                    fo = gl * 2 * C + ti * C
                    nc.tensor.matmul(
                        acc[cg][sl[cg], :],
                        ohg,
                        nt[:, fo : fo + C],
                        start=(g == par),
                        stop=(g == NG - 2 + par),
                        tile_position=(0, 32 * cg),
                    )
                g += 1
                if g == NG - 1:
                    # Even-pair accumulators just stopped: drain + store
                    # them now (one wide ACT copy spanning both 19-row
                    # bands), overlapping the final group's DMA/matmuls.
                    # ob is fp16 (copies cast): halves the out-DMA bytes.
                    nc.scalar.copy(ob[0:51, :], acc01[0:51, :])
                    nc.sync.dma_start(out[0:64, :], ob[0:64, :])
        # Odd-pair drain right after the final matmuls: one wide DVE
        # copy (0.7us, the critical tail), then one [64,512] fp16 DMA.
        nc.vector.tensor_copy(ob[64:115, :], acc23[64:115, :])
        nc.scalar.dma_start(out[64:128, :], ob[64:128, :])
    nc.finalize()
    return nc


_NC_CACHE = None


def _get_nc():
    global _NC_CACHE
    if _NC_CACHE is None:
        _NC_CACHE = build_nc()
    return _NC_CACHE


def _pack_core(fs_i, ft_i, lab_i):
    """Pack one batch element: fp8 pixel-major feature image + f32
    iota|labels side tensor (partition = pixel % 128)."""
    sT = fs_i.reshape(C, NG, P).astype(QDT_NP).transpose(2, 1, 0)  # [P, NG, C]
    tT = ft_i.reshape(C, NG, P).astype(QDT_NP).transpose(2, 1, 0)
    img = np.zeros((FEAT_ROWS, FEAT_STRIDE), QDT_NP)
    img[PAD_ROWS : PAD_ROWS + P, :FEAT_W] = np.stack([sT, tT], axis=2).reshape(
        P, FEAT_W
    )

    labi = np.empty((P, LABW), np.float32)
    labi[:, :NUM_CLASSES] = np.arange(NUM_CLASSES, dtype=np.float32)
    labi[:, NUM_CLASSES:] = lab_i.reshape(NG, P).T.astype(np.float32)
    return {"feat": img, "labi": labi}


def _make_in_maps(features_s, features_t, labels):
    return [
        _pack_core(features_s[i], features_t[i], labels[i].reshape(-1))
        for i in range(N_CORES)
    ]


def _finish_on_host(results, labels):
    S_s = np.zeros((NUM_CLASSES, C), np.float64)
    S_t = np.zeros((NUM_CLASSES, C), np.float64)
    for r in results:
        o = r["sums"]
        S_s += o[0:NUM_CLASSES]
        S_s += o[64 : 64 + NUM_CLASSES]
        S_t += o[32 : 32 + NUM_CLASSES]
        S_t += o[96 : 96 + NUM_CLASSES]
    counts = np.bincount(
        labels.reshape(-1), minlength=NUM_CLASSES
    ).astype(np.float64)
    denom = np.maximum(counts, 1.0)[:, None]

    def l2n(x):
        n = np.linalg.norm(x, axis=1, keepdims=True)
        return x / np.maximum(n, EPS)

    logits = (l2n(S_s / denom) @ l2n(S_t / denom).T) / TEMP
    m = logits.max(axis=1, keepdims=True)
    lse = m[:, 0] + np.log(np.exp(logits - m).sum(axis=1))
    per_class = np.diag(logits) - lse
    present = counts > 0
    loss = -np.sum(np.where(present, per_class, 0.0)) / np.sum(present)
    return np.asarray(loss, dtype=np.float32)


def kernel(features_s, features_t, labels, _trace=False):
    features_s = np.asarray(features_s, dtype=np.float32)
    features_t = np.asarray(features_t, dtype=np.float32)
    labels = np.asarray(labels)
    nc = _get_nc()
    in_maps = _make_in_maps(features_s, features_t, labels)
    res = run_bass_kernel_spmd(nc, in_maps, list(range(N_CORES)), trace=_trace)
    loss = _finish_on_host(res.results, labels)
    if _trace:
        return loss, res
    return loss
